# revision 15
# baseline (speedup 1.0000x reference)
"""Multi-head self-attention (B=4, N=2048, D=1024, H=16) on 8 trn2 NeuronCores.

Sharding: 8 shards = (batch, query-half).  Core c handles batch c//2 and query
rows [(c%2)*1024, (c%2)*1024+1024).  Each core receives its batch's z with the
rows rolled so that its query rows come first; rolling permutes the key/value
sequence order, which attention output is invariant to.  K/V are computed for
the full 2048-row sequence on both cores of a batch pair.

Single fused phase, software-pipelined by head pair (hp) so that ACT (exp)
overlaps the next head pair's projections:
  - z^T via PE transposes into a resident bf16 ztc [128, 8, 2048].
  - Per hp: K^T/Q^T projections (bf16 weights, host-cast) into ping-pong SBUF
    slots; V' = [V|1] (all heads) woven into the early hp windows.
  - Per head (sequential): scores via 64-contraction matmuls, exp on ACT
    ([128,1024] activates), PV with the ones column producing the softmax
    denominator in row 64.  PV accumulates 8 key-chunks in PSUM, then is
    flushed/accumulated into an SBUF fp32 tile to keep PSUM pressure at
    2 banks (8 total: scores 2x2 + pv 2 + proj 2).
  - Normalization: reciprocal of the denominator row + gpsimd partition
    broadcast + DVE multiply into bf16 attnT.
  - Final projection attnT^T @ w_o + b_o at the tail through recycled PSUM.
"""

import os
import sys

_TRN_REPO = "/opt/trn_rl_repo"
if os.path.isdir(_TRN_REPO) and _TRN_REPO not in sys.path:
    sys.path.insert(0, _TRN_REPO)

import ml_dtypes
import numpy as np

import concourse.bass as bass  # noqa: E402
import concourse.mybir as mybir  # noqa: E402
from concourse import bacc  # noqa: E402
from concourse.bass_utils import run_bass_kernel_spmd  # noqa: E402
from concourse.masks import make_identity  # noqa: E402
from concourse.tile import TileContext  # noqa: E402

F32 = mybir.dt.float32
BF16 = mybir.dt.bfloat16
MULT = mybir.AluOpType.mult
ADD = mybir.AluOpType.add
EXP = mybir.ActivationFunctionType.Exp

N_CORES = 8
B, N, D = 4, 2048, 1024
H, HD = 16, 64
NQ = N // 2  # query rows per core
P = 128
DC = D // P  # 8 din/dout chunks of 128
NKC = N // P  # 16 key chunks of 128
HP = H // 2  # 8 head pairs
SCALE = 1.0 / 8.0  # 1/sqrt(HD)


def _build():
    nc = bacc.Bacc("TRN2", target_bir_lowering=False, debug=False,
                   num_devices=N_CORES)
    z_d = nc.declare_dram_parameter("z", [N, D], BF16, isOutput=False)
    wq_d = nc.declare_dram_parameter("w_q", [D, D], BF16, isOutput=False)
    wk_d = nc.declare_dram_parameter("w_k", [D, D], BF16, isOutput=False)
    wv_d = nc.declare_dram_parameter("w_v", [D, D], BF16, isOutput=False)
    wo_d = nc.declare_dram_parameter("w_o", [D, D], BF16, isOutput=False)
    bo_d = nc.declare_dram_parameter("b_o", [D], F32, isOutput=False)
    out_d = nc.declare_dram_parameter("out", [NQ, D], F32, isOutput=True)

    with TileContext(nc) as tc:
        with tc.tile_pool(name="wp", bufs=1) as wp, \
             tc.tile_pool(name="zt", bufs=1) as ztp, \
             tc.tile_pool(name="kq", bufs=1) as kqp, \
             tc.tile_pool(name="vpool", bufs=1) as vpool, \
             tc.tile_pool(name="es", bufs=6) as esp, \
             tc.tile_pool(name="at", bufs=1) as atp, \
             tc.tile_pool(name="rr", bufs=1) as rrp, \
             tc.tile_pool(name="ot", bufs=2) as otp, \
             tc.tile_pool(name="psc", bufs=2, space="PSUM") as pscp, \
             tc.tile_pool(name="ppv", bufs=1, space="PSUM") as ppvp, \
             tc.tile_pool(name="pac", bufs=2, space="PSUM") as pacp:

            # ---- constants & resident tensors ----
            wk_sb = wp.tile([P, DC, D], BF16)
            nc.scalar.dma_start(wk_sb[:], wk_d.rearrange("(c p) o -> p c o", p=P))
            wq_sb = wp.tile([P, DC, D], BF16)
            nc.scalar.dma_start(wq_sb[:], wq_d.rearrange("(c p) o -> p c o", p=P))
            wv_sb = wp.tile([P, DC, D], BF16)
            nc.scalar.dma_start(wv_sb[:], wv_d.rearrange("(c p) o -> p c o", p=P))
            wo_sb = wp.tile([P, DC, D], BF16)
            nc.scalar.dma_start(wo_sb[:], wo_d.rearrange("(c p) o -> p c o", p=P))
            bo_sb = wp.tile([1, D], F32)
            nc.scalar.dma_start(bo_sb[:], bo_d[None, :])
            bo_bc = wp.tile([P, D], F32)
            nc.gpsimd.partition_broadcast(bo_bc[:], bo_sb[:])

            ztc = ztp.tile([P, DC, N], BF16)            # z^T, din-major
            ktz = kqp.tile([P, 2, N], BF16)             # K^T hp ping-pong
            qtz = kqp.tile([P, 2, NQ], BF16)            # Q^T hp ping-pong
            vp = vpool.tile([P, NKC, H, HD + 1], BF16)  # V' = [V | 1]
            nc.vector.memset(vp[:, :, :, HD], 1.0)
            attnT = atp.tile([P, DC, NQ], BF16)         # normalized attn^T

            # ---- z^T via XBAR transpose-DMA (z is bf16, host-cast) ----
            for s2 in range(2):
                for dc in range(DC):
                    nc.sync.dma_start_transpose(
                        ztc[:, dc, s2 * 1024:(s2 + 1) * 1024],
                        z_d[s2 * 1024:(s2 + 1) * 1024, dc * P:(dc + 1) * P])

            # ---- projection chunk helpers ----
            def k_chunk(hp, s):
                ps = pacp.tile([P, 512], F32, tag="acc")
                for dc in range(DC):
                    nc.tensor.matmul(
                        ps[:],
                        lhsT=wk_sb[:, dc, hp * P:(hp + 1) * P],
                        rhs=ztc[:, dc, s * 512:(s + 1) * 512],
                        start=(dc == 0), stop=(dc == DC - 1))
                nc.vector.tensor_copy(ktz[:, hp % 2, s * 512:(s + 1) * 512], ps[:])

            def q_chunk(hp, s):
                ps = pacp.tile([P, 512], F32, tag="acc")
                for dc in range(DC):
                    nc.tensor.matmul(
                        ps[:],
                        lhsT=wq_sb[:, dc, hp * P:(hp + 1) * P],
                        rhs=ztc[:, dc, s * 512:(s + 1) * 512],
                        start=(dc == 0), stop=(dc == DC - 1))
                nc.vector.tensor_copy(qtz[:, hp % 2, s * 512:(s + 1) * 512], ps[:])

            def v_chunk(kc4, oc2):
                ps = pacp.tile([P, 512], F32, tag="acc")
                for dc in range(DC):
                    nc.tensor.matmul(
                        ps[:],
                        lhsT=ztc[:, dc, kc4 * P:(kc4 + 1) * P],
                        rhs=wv_sb[:, dc, oc2 * 512:(oc2 + 1) * 512],
                        start=(dc == 0), stop=(dc == DC - 1))
                nc.vector.tensor_copy(
                    vp[:, kc4, oc2 * 8:(oc2 + 1) * 8, 0:HD],
                    ps.rearrange("p (h d) -> p h d", d=HD))

            # ---- lead-in: hp0's K^T/Q^T ----
            for s in range(4):
                k_chunk(0, s)
            for s in range(2):
                q_chunk(0, s)

            # V chunks for oc2=1 (heads 8-15), woven into hp1-3 windows
            v1_list = [(kc4, 1) for kc4 in range(NKC)]
            v1_split = {1: v1_list[0:6], 2: v1_list[6:11], 3: v1_list[11:16]}

            # ---- main loop over head pairs, heads sequential ----
            for hp in range(HP):
                slot = hp % 2

                fillers = []
                if hp + 1 < HP:
                    for s in range(4):
                        fillers.append(lambda s=s, n=hp + 1: k_chunk(n, s))
                    for s in range(2):
                        fillers.append(lambda s=s, n=hp + 1: q_chunk(n, s))
                for kc4, oc2 in v1_split.get(hp, []):
                    fillers.append(lambda a=kc4, b=oc2: v_chunk(a, b))

                for head in range(2):
                    h = 2 * hp + head
                    po = 64 * head
                    pv = ppvp.tile([HD + 1, NQ], F32, tag="pv")
                    es_hist = {}

                    def emit_pv(kc, h=h, pv=pv, es_hist=es_hist):
                        es = es_hist.pop(kc)
                        for qc in range(2):
                            nc.tensor.matmul(
                                pv[:, qc * 512:(qc + 1) * 512],
                                lhsT=vp[:, kc, h, :],
                                rhs=es[:, qc * 512:(qc + 1) * 512],
                                start=(kc == 0), stop=(kc == NKC - 1))

                    # software-pipelined: PV lags scores/exp by 2 key-chunks
                    # so the PE never head-of-line blocks on ACT.
                    for kc in range(NKC):
                        if hp == 0 and head == 0:
                            v_chunk(kc, 0)
                        if kc >= 2:
                            emit_pv(kc - 2)
                        if kc % 2 == 1 and fillers:
                            fillers.pop(0)()
                        ps = pscp.tile([P, NQ], F32, tag="sc")
                        for qc in range(2):
                            nc.tensor.matmul(
                                ps[:, qc * 512:(qc + 1) * 512],
                                lhsT=ktz[po:po + 64, slot, kc * P:(kc + 1) * P],
                                rhs=qtz[po:po + 64, slot, qc * 512:(qc + 1) * 512])
                        es = esp.tile([P, NQ], BF16, tag="es")
                        nc.scalar.activation(es[:], ps[:], EXP, scale=SCALE)
                        es_hist[kc] = es
                    emit_pv(NKC - 2)
                    emit_pv(NKC - 1)

                    # normalization straight from PSUM
                    rec = rrp.tile([1, NQ], F32, tag="rec")
                    nc.vector.reciprocal(rec[:], pv[HD:HD + 1, :])
                    rb = rrp.tile([64, NQ], F32, tag="rb")
                    nc.gpsimd.partition_broadcast(rb[:], rec[:])
                    nc.vector.tensor_tensor(
                        attnT[po:po + 64, hp, :],
                        pv[0:HD, :], rb[:], MULT)
                while fillers:
                    fillers.pop(0)()

            # ---- tail: out = attnT^T @ w_o + b_o ----
            for q8 in range(NQ // P):
                if q8 % 3 == 2:
                    ps = ppvp.tile([P, NQ], F32, tag="pv")
                else:
                    ps = pscp.tile([P, NQ], F32, tag="sc")
                for oc2 in range(2):
                    for dc in range(DC):
                        nc.tensor.matmul(
                            ps[:, oc2 * 512:(oc2 + 1) * 512],
                            lhsT=attnT[:, dc, q8 * P:(q8 + 1) * P],
                            rhs=wo_sb[:, dc, oc2 * 512:(oc2 + 1) * 512],
                            start=(dc == 0), stop=(dc == DC - 1))
                for oc2 in range(2):
                    ot = otp.tile([P, 512], F32, tag="ot")
                    nc.vector.tensor_tensor(
                        ot[:], ps[:, oc2 * 512:(oc2 + 1) * 512],
                        bo_bc[:, oc2 * 512:(oc2 + 1) * 512], ADD)
                    nc.sync.dma_start(
                        out_d[q8 * P:(q8 + 1) * P,
                              oc2 * 512:(oc2 + 1) * 512], ot[:])

    nc.compile()
    return nc


_NC_CACHE = None


def _get_nc():
    global _NC_CACHE
    if _NC_CACHE is None:
        _NC_CACHE = _build()
    return _NC_CACHE


def _run(z, w_q, w_k, w_v, w_o, b_o, **spmd_kwargs):
    z = np.ascontiguousarray(np.asarray(z, dtype=np.float32)).astype(
        ml_dtypes.bfloat16)
    w_q = np.ascontiguousarray(np.asarray(w_q, dtype=np.float32)).astype(
        ml_dtypes.bfloat16)
    w_k = np.ascontiguousarray(np.asarray(w_k, dtype=np.float32)).astype(
        ml_dtypes.bfloat16)
    w_v = np.ascontiguousarray(np.asarray(w_v, dtype=np.float32)).astype(
        ml_dtypes.bfloat16)
    w_o = np.ascontiguousarray(np.asarray(w_o, dtype=np.float32)).astype(
        ml_dtypes.bfloat16)
    b_o = np.ascontiguousarray(np.asarray(b_o, dtype=np.float32))
    assert z.shape == (B, N, D)

    if not spmd_kwargs.get("trace"):
        # A stray BASS_TRACE in the environment would route through the NTFF
        # hook (absent in this image) and crash; force the no-trace path.
        os.environ["BASS_NEVER_TRACE"] = "1"

    nc = _get_nc()
    in_maps = []
    for c in range(N_CORES):
        b = c // 2
        off = (c % 2) * NQ
        zc = np.ascontiguousarray(np.concatenate([z[b, off:], z[b, :off]], axis=0))
        in_maps.append({"z": zc, "w_q": w_q, "w_k": w_k, "w_v": w_v,
                        "w_o": w_o, "b_o": b_o})

    res = run_bass_kernel_spmd(nc, in_maps, core_ids=list(range(N_CORES)),
                               **spmd_kwargs)
    out = np.empty((B, N, D), dtype=np.float32)
    for c in range(N_CORES):
        b = c // 2
        off = (c % 2) * NQ
        out[b, off:off + NQ, :] = res.results[c]["out"]
    return out, res


def kernel(z, w_q, w_k, w_v, w_o, b_o):
    out, _ = _run(z, w_q, w_k, w_v, w_o, b_o)
    return out


# revision 16
# speedup vs baseline: 1.2575x; 1.2575x over previous
"""Multi-head self-attention (B=4, N=2048, D=1024, H=16) on 8 trn2 NeuronCores.

Sharding: 8 shards = (batch, query-half).  Core c handles batch c//2 and query
rows [(c%2)*1024, (c%2)*1024+1024).  Each core receives its batch's z with the
rows rolled so that its query rows come first; rolling permutes the key/value
sequence order, which attention output is invariant to.  K/V are computed for
the full 2048-row sequence on both cores of a batch pair (duplicated compute,
no collectives needed).

Per-core kernel (Tile):
  1. PE-transpose z -> zT (din-major), fp32.
  2. Q^T/K^T (d-major) and V (natural, with a ones column appended per head)
     projections via float32r matmuls; K^T/Q^T spilled to DRAM scratch.
  3. Per head: scores S^T = K Q^T (f32r), exp(s/8) on ACT -> bf16,
     P^T@V via matmul with V|ones (denominator accumulates in row 64),
     reciprocal + gpsimd partition-broadcast, normalized attn^T in fp32.
  4. Final projection attn @ w_o + b_o in f32r, bias via partition-broadcast.
"""

import os
import sys

_TRN_REPO = "/opt/trn_rl_repo"
if os.path.isdir(_TRN_REPO) and _TRN_REPO not in sys.path:
    sys.path.insert(0, _TRN_REPO)

import numpy as np

import concourse.bass as bass  # noqa: E402
import concourse.mybir as mybir  # noqa: E402
from concourse import bacc  # noqa: E402
from concourse.bass_utils import run_bass_kernel_spmd  # noqa: E402
from concourse.masks import make_identity  # noqa: E402
from concourse.tile import TileContext  # noqa: E402

F32 = mybir.dt.float32
F32R = mybir.dt.float32r
BF16 = mybir.dt.bfloat16
MULT = mybir.AluOpType.mult
ADD = mybir.AluOpType.add
EXP = mybir.ActivationFunctionType.Exp

N_CORES = 8
B, N, D = 4, 2048, 1024
H, HD = 16, 64
NQ = N // 2  # query rows per core
P = 128
DC = D // P  # 8 din/dout chunks of 128
NKC = N // P  # 16 key chunks of 128
SCALE = 1.0 / 8.0  # 1/sqrt(HD)


def _build():
    nc = bacc.Bacc("TRN2", target_bir_lowering=False, debug=False,
                   num_devices=N_CORES)
    z_d = nc.declare_dram_parameter("z", [N, D], F32, isOutput=False)
    wq_d = nc.declare_dram_parameter("w_q", [D, D], F32R, isOutput=False)
    wk_d = nc.declare_dram_parameter("w_k", [D, D], F32R, isOutput=False)
    wv_d = nc.declare_dram_parameter("w_v", [D, D], F32R, isOutput=False)
    wo_d = nc.declare_dram_parameter("w_o", [D, D], F32R, isOutput=False)
    bo_d = nc.declare_dram_parameter("b_o", [D], F32, isOutput=False)
    out_d = nc.declare_dram_parameter("out", [NQ, D], F32, isOutput=True)

    # DRAM scratch: K^T/Q^T in partition-major layout for clean reload.
    kts_d = nc.dram_tensor("kts", [P, DC, N], BF16)
    qts_d = nc.dram_tensor("qts", [P, DC, NQ], BF16)

    with TileContext(nc) as tc:
        with tc.tile_pool(name="const", bufs=1) as constp, \
             tc.tile_pool(name="vpool", bufs=1) as vpool:
            ident = constp.tile([P, P], F32)
            make_identity(nc, ident)
            # V' = [V_h | 1] per head: [P, key-chunk, head, 65] bf16
            vp = vpool.tile([P, NKC, H, HD + 1], BF16)
            nc.vector.memset(vp[:, :, :, HD], 1.0)
            # K^T/Q^T zero-padded scores operands live OUTSIDE the phase
            # pools so their zero rows are written at t=0 and phase-2 has no
            # SBUF zone handoff before the first scores matmul.
            ktz = vpool.tile([P, 2, N], BF16)
            qtz = vpool.tile([P, 2, NQ], BF16)
            nc.vector.memset(ktz[64:P, :, :], 0.0)
            nc.vector.memset(qtz[64:P, :, :], 0.0)

            # ---------------- Phase 1: zT + projections ----------------
            with tc.tile_pool(name="zin", bufs=1) as zinp, \
                 tc.tile_pool(name="zt", bufs=2) as ztp, \
                 tc.tile_pool(name="wt", bufs=3) as wtp, \
                 tc.tile_pool(name="stg", bufs=3) as stgp, \
                 tc.tile_pool(name="pst", bufs=2, space="PSUM") as pst, \
                 tc.tile_pool(name="psp", bufs=6, space="PSUM") as psp:

                zt_first = zinp.tile([P, 4, D], F32, name="zt_in")
                nc.sync.dma_start(
                    zt_first[:],
                    z_d[0:512, :].rearrange("(r p) d -> p r d", p=P))
                wk_sb = wtp.tile([P, DC, D], F32R, tag="w")
                nc.scalar.dma_start(wk_sb[:], wk_d.rearrange("(c p) o -> p c o", p=P))
                wq_sb = wtp.tile([P, DC, D], F32R, tag="w")
                nc.scalar.dma_start(wq_sb[:], wq_d.rearrange("(c p) o -> p c o", p=P))
                wv_sb = wtp.tile([P, DC, D], F32R, tag="w")
                nc.scalar.dma_start(wv_sb[:], wv_d.rearrange("(c p) o -> p c o", p=P))

                for n5 in range(N // 512):  # 4 big chunks of 512 seq rows
                    # transpose 512 z rows -> ztc [P, DC, 512]
                    ztc = ztp.tile([P, DC, 512], F32R)
                    if n5 == 0:
                        zt_in = zt_first
                    else:
                        zt_in = zinp.tile([P, 4, D], F32, name="zt_in")
                        nc.sync.dma_start(
                            zt_in[:],
                            z_d[n5 * 512:(n5 + 1) * 512, :].rearrange(
                                "(r p) d -> p r d", p=P))
                    for dc in range(DC):
                        ps = pst.tile([P, 512], F32)
                        for r in range(4):
                            nc.tensor.transpose(
                                ps[:, r * P:(r + 1) * P],
                                zt_in[:, r, dc * P:(dc + 1) * P],
                                ident[:])
                        nc.vector.tensor_copy(ztc[:, dc, :], ps[:])

                    # K^T chunk: [dout, 512] for all 8 dout chunks
                    for og in range(2):
                        pss = [psp.tile([P, 512], F32, name="pp") for _ in range(4)]
                        for dc in range(DC):
                            for j in range(4):
                                oc = og * 4 + j
                                nc.tensor.matmul(
                                    pss[j][:],
                                    lhsT=(wk_sb[:, dc, oc * P:(oc + 1) * P]),
                                    rhs=(ztc[:, dc, :]),
                                    start=(dc == 0), stop=(dc == DC - 1))
                        for j in range(4):
                            st = stgp.tile([P, 512], BF16)
                            nc.vector.tensor_copy(st[:], pss[j][:])
                            nc.scalar.dma_start(
                                kts_d[:, og * 4 + j, n5 * 512:(n5 + 1) * 512], st[:])

                    # Q^T chunk (first 1024 rows only)
                    if n5 < NQ // 512:
                        for og in range(2):
                            pss = [psp.tile([P, 512], F32, name="pp") for _ in range(4)]
                            for dc in range(DC):
                                for j in range(4):
                                    oc = og * 4 + j
                                    nc.tensor.matmul(
                                        pss[j][:],
                                        lhsT=(wq_sb[:, dc, oc * P:(oc + 1) * P]),
                                        rhs=(ztc[:, dc, :]),
                                        start=(dc == 0), stop=(dc == DC - 1))
                            for j in range(4):
                                st = stgp.tile([P, 512], BF16)
                                nc.vector.tensor_copy(st[:], pss[j][:])
                                nc.scalar.dma_start(
                                    qts_d[:, og * 4 + j, n5 * 512:(n5 + 1) * 512],
                                    st[:])

                    # V chunk: natural [k, dout] -> V' (strided per head)
                    for kcp in range(2):
                        pss = [psp.tile([P, 512], F32, name="pp") for _ in range(4)]
                        for dc in range(DC):
                            for i2 in range(2):
                                kc4 = kcp * 2 + i2
                                lh = (ztc[:, dc, kc4 * P:(kc4 + 1) * P])
                                for oc2 in range(2):
                                    nc.tensor.matmul(
                                        pss[i2 * 2 + oc2][:],
                                        lhsT=lh,
                                        rhs=(wv_sb[:, dc, oc2 * 512:(oc2 + 1) * 512]),
                                        start=(dc == 0), stop=(dc == DC - 1))
                        for i2 in range(2):
                            kcg = n5 * 4 + kcp * 2 + i2
                            for oc2 in range(2):
                                nc.vector.tensor_copy(
                                    vp[:, kcg, oc2 * 8:(oc2 + 1) * 8, 0:HD],
                                    pss[i2 * 2 + oc2].rearrange(
                                        "p (h d) -> p h d", d=HD))

            # ---------------- Phases 2+3 ----------------
            with tc.tile_pool(name="at", bufs=1) as atp:
                attnT = atp.tile([P, DC, NQ], BF16)
                bo_sb = atp.tile([1, D], F32)
                nc.scalar.dma_start(bo_sb[:], bo_d[None, :])
                bo_bc = atp.tile([P, D], F32)
                nc.gpsimd.partition_broadcast(bo_bc[:], bo_sb[:])
                wo_sb = atp.tile([P, DC, D], F32R)
                nc.scalar.dma_start(wo_sb[:], wo_d.rearrange("(c p) o -> p c o", p=P))
                wo16 = atp.tile([P, DC, D], BF16)
                nc.vector.tensor_copy(wo16[:], wo_sb[:])

                # Phase 2: attention per head
                # K^T/Q^T zero-padded to 128 contraction rows (rows 64-127 = 0)
                # so scores matmuls use the full PE array (keeps HAM warm).
                with tc.tile_pool(name="es", bufs=8) as esp, \
                     tc.tile_pool(name="rc", bufs=4) as recp, \
                     tc.tile_pool(name="pss", bufs=2, space="PSUM") as ssp, \
                     tc.tile_pool(name="pvo", bufs=4, space="PSUM") as pvp:
                    for h in range(H):
                        bf = h % 2
                        po = 64 * (h % 2)
                        nc.sync.dma_start(ktz[0:64, bf, :],
                                          kts_d[po:po + 64, h // 2, :])
                        nc.sync.dma_start(qtz[0:64, bf, :],
                                          qts_d[po:po + 64, h // 2, :])
                        pso = [pvp.tile([P, 512], F32, name="pvo") for _ in range(2)]
                        for kc in range(NKC):
                            ps = ssp.tile([P, NQ], F32, name="pss")
                            es = esp.tile([P, NQ], BF16)
                            for qc in range(2):
                                nc.tensor.matmul(
                                    ps[:, qc * 512:(qc + 1) * 512],
                                    lhsT=ktz[:, bf, kc * P:(kc + 1) * P],
                                    rhs=qtz[:, bf, qc * 512:(qc + 1) * 512])
                            nc.scalar.activation(es[:], ps[:], EXP, scale=SCALE)
                            lh = vp[:, kc, h, :]
                            for qc in range(2):
                                nc.tensor.matmul(
                                    pso[qc][0:HD + 1, :],
                                    lhsT=lh,
                                    rhs=es[:, qc * 512:(qc + 1) * 512],
                                    start=(kc == 0), stop=(kc == NKC - 1))
                        for qc in range(2):
                            rec = recp.tile([1, 512], F32, tag="rec")
                            nc.vector.reciprocal(rec[:], pso[qc][HD:HD + 1, :])
                            rb = recp.tile([64, 512], F32, tag="rb")
                            nc.gpsimd.partition_broadcast(rb[:], rec[:])
                            nc.vector.tensor_tensor(
                                attnT[po:po + 64, h // 2, qc * 512:(qc + 1) * 512],
                                pso[qc][0:HD, :], rb[:], MULT)

                # Phase 3: final projection + bias
                with tc.tile_pool(name="ot", bufs=4) as outp, \
                     tc.tile_pool(name="psf", bufs=4, space="PSUM") as fpp:
                    for q8 in range(NQ // P):
                        psf = [fpp.tile([P, 512], F32, name="pf") for _ in range(2)]
                        for dc in range(DC):
                            lh = (attnT[:, dc, q8 * P:(q8 + 1) * P])
                            for oc2 in range(2):
                                nc.tensor.matmul(
                                    psf[oc2][:],
                                    lhsT=lh,
                                    rhs=wo16[:, dc, oc2 * 512:(oc2 + 1) * 512],
                                    start=(dc == 0), stop=(dc == DC - 1))
                        for oc2 in range(2):
                            ot = outp.tile([P, 512], F32)
                            nc.vector.tensor_tensor(
                                ot[:], psf[oc2][:],
                                bo_bc[:, oc2 * 512:(oc2 + 1) * 512], ADD)
                            nc.sync.dma_start(
                                out_d[q8 * P:(q8 + 1) * P,
                                      oc2 * 512:(oc2 + 1) * 512], ot[:])

    nc.compile()
    return nc


_NC_CACHE = None


def _get_nc():
    global _NC_CACHE
    if _NC_CACHE is None:
        _NC_CACHE = _build()
    return _NC_CACHE


def _run(z, w_q, w_k, w_v, w_o, b_o, **spmd_kwargs):
    z = np.ascontiguousarray(np.asarray(z, dtype=np.float32))
    w_q = np.ascontiguousarray(np.asarray(w_q, dtype=np.float32))
    w_k = np.ascontiguousarray(np.asarray(w_k, dtype=np.float32))
    w_v = np.ascontiguousarray(np.asarray(w_v, dtype=np.float32))
    w_o = np.ascontiguousarray(np.asarray(w_o, dtype=np.float32))
    b_o = np.ascontiguousarray(np.asarray(b_o, dtype=np.float32))
    assert z.shape == (B, N, D)

    if not spmd_kwargs.get("trace"):
        # A stray BASS_TRACE in the environment would route through the NTFF
        # hook (absent in this image) and crash; force the no-trace path.
        os.environ["BASS_NEVER_TRACE"] = "1"

    nc = _get_nc()
    in_maps = []
    for c in range(N_CORES):
        b = c // 2
        off = (c % 2) * NQ
        zc = np.ascontiguousarray(np.concatenate([z[b, off:], z[b, :off]], axis=0))
        in_maps.append({"z": zc, "w_q": w_q, "w_k": w_k, "w_v": w_v,
                        "w_o": w_o, "b_o": b_o})

    res = run_bass_kernel_spmd(nc, in_maps, core_ids=list(range(N_CORES)),
                               **spmd_kwargs)
    out = np.empty((B, N, D), dtype=np.float32)
    for c in range(N_CORES):
        b = c // 2
        off = (c % 2) * NQ
        out[b, off:off + NQ, :] = res.results[c]["out"]
    return out, res


def kernel(z, w_q, w_k, w_v, w_o, b_o):
    out, _ = _run(z, w_q, w_k, w_v, w_o, b_o)
    return out

